# revision 17
# baseline (speedup 1.0000x reference)
"""CapsuleLayer (dynamic routing, 3 iterations) Trainium2 Bass kernel — v3.

Full inputs:  input_vectors [32, 2048, 16] f32, weight_matrix [1, 64, 32, 16] f32
Full output:  [32, 64, 32] f32

Sharding: data-parallel over batch; each of 8 NeuronCores processes 4 batches.
No collectives.

v3 changes vs v2 (67.6us):
  - ZERO act-table switches: sqrt runs on GpSimd via tensor_scalar(pow, 0.5);
    the scalar engine only ever uses Exp (+Copy), all in one resident table.
    (v2 paid 9 x 1283ns ACT_TABLE_LOADs for the ln/exp-based sqrt.)
  - x is cast to fp16 on host: half the DMA bytes, and 2-byte DVE ops.
  - x DMA issued first; all other constants packed into ONE dram tensor.
  - squash square+reduce on DVE (fp16, 2x modes) instead of scalar SQUARE.
  - squashed x written directly into the padded/permuted xsp layout (no
    separate pad-copy); transposes batched 4-per-PSUM-tile with single copies.
  - logits matmuls batch-paired via block-diag rhs: 32 MMs of 128 cols per
    iteration instead of 64, chunked 4 strips at a time over PSUM so EXP and
    the Z-reduce pipeline behind the MMs.
  - Z-reduce / reciprocal with fp16 outputs (DVE 2x path).
  - t-matmul streams all 4 batches per (strip, pair) MM: 32 MMs of 64 cols.
  - trc (wv^T blocks) built with 2 zeroing transposes + 4 block transposes +
    one PSUM->SBUF copy.
  - output written with a single fused DMA.

n-to-SBUF mapping: n = p*16 + j  (p = partition, j = strip 0..15).
t/psum layout: tps2 [128, (pr, b, i)]; valid rows: b even 0:64, b odd 64:128.
small stage in [128-part, (z, i)]: rows 0:64 hold (b0,b2), 64:128 (b1,b3).
Iteration-2 logits rhs built from wv0+wv1 (linearity) -> no PSUM carry-over.
"""

import os

os.environ.setdefault("MYCRO_LOCAL_CACHE", "1")

import numpy as np

import concourse.bass as bass
import concourse.tile as tile
from concourse import bacc, mybir
from concourse.bass_utils import run_bass_kernel_spmd

AF = mybir.ActivationFunctionType
ALU = mybir.AluOpType
F32 = mybir.dt.float32
F16 = mybir.dt.float16

N_CORES = 8
B = 4          # batches per core
N = 2048       # input capsules
O = 64         # output capsules
DI = 16        # input capsule dim
D = 32         # output capsule dim
J = 16         # n-strips per batch (n = p*16 + j)
EPS = 0.5

WARMUP_MMS = int(os.environ.get("CAPS_WARMUP_MMS", "60"))
SQRT_MODE = os.environ.get("CAPS_SQRT", "gpstt")
LOGITS_MODE = os.environ.get("CAPS_LOGITS", "pair")
DEBUG_DUMP = os.environ.get("CAPS_DEBUG_DUMP", "")


def build_kernel(nc: bass.Bass, tc: tile.TileContext):
    from contextlib import ExitStack
    ctx = ExitStack()
    x = nc.dram_tensor("x", [B, N, DI], F16, kind="ExternalInput").ap()
    cpack = nc.dram_tensor("cpack", [128, 896], F16, kind="ExternalInput").ap()
    vout = nc.dram_tensor("vout", [B, O, D], F32, kind="ExternalOutput").ap()

    const = ctx.enter_context(tc.tile_pool(name="const", bufs=1))
    big = ctx.enter_context(tc.tile_pool(name="big", bufs=1))
    small = ctx.enter_context(tc.tile_pool(name="small", bufs=2))
    psumT = ctx.enter_context(tc.tile_pool(name="psumT", bufs=1, space="PSUM"))
    psumL = ctx.enter_context(tc.tile_pool(name="psumL", bufs=2, space="PSUM"))
    psumX = ctx.enter_context(tc.tile_pool(name="psumX", bufs=2, space="PSUM"))
    psumW = ctx.enter_context(tc.tile_pool(name="psumW", bufs=1, space="PSUM"))

    def squash_scale(out, n2, tag):
        # out = sqrt(n2)/(eps+n2).  sqrt on GpSimd (tensor_tensor pow — the
        # only pow the Pool engine ISA accepts) in parallel with the
        # add+reciprocal on DVE; no scalar-engine act tables involved.
        s = small.tile(list(n2.shape), F32, tag=f"{tag}_s")
        cols = n2.shape[1]
        if SQRT_MODE == "dvets":
            nc.vector.tensor_scalar(s[:], n2, 0.5, None, op0=ALU.pow)
        elif SQRT_MODE == "act":
            ln2 = small.tile(list(n2.shape), F32, tag=f"{tag}_ln2")
            nc.scalar.activation(ln2[:], n2, AF.Ln)
            nc.scalar.activation(s[:], ln2[:], AF.Exp, 0.0, 0.5)
        else:
            nc.gpsimd.tensor_tensor(s[:], n2, half_sb[:, 0:cols], op=ALU.pow)
        d = small.tile(list(n2.shape), F32, tag=f"{tag}_d")
        nc.vector.tensor_scalar_add(d[:], n2, EPS)
        rd = small.tile(list(n2.shape), F32, tag=f"{tag}_rd")
        nc.vector.reciprocal(rd[:], d[:])
        nc.vector.tensor_mul(out, s[:], rd[:])

    def dump_stop(src, note=""):
        # stage src ([P, C] any dtype, P<=128, C<=64) into f32 and write vout
        stage = const.tile([128, 64], F32, tag="dumpstage")
        nc.gpsimd.memset(stage[:], 0.0)
        P, C = src.shape[0], src.shape[1]
        nc.vector.tensor_copy(stage[0:P, 0:C], src)
        nc.sync.dma_start(
            vout.rearrange("(z b2) o d -> (b2 o) z d", z=2),
            stage[:].rearrange("p (z d) -> p z d", z=2),
        )
        ctx.close()

    # ---- input DMA first (largest transfer), then constants ----
    xr = big.tile([128, B * J * DI], F16, tag="xr")
    nc.sync.dma_start(
        xr[:].rearrange("p (b j i) -> p b j i", b=B, j=J),
        x.rearrange("b (p j) i -> p b j i", p=128),
    )
    cpk = const.tile([128, 896], F16, tag="cpk")
    nc.sync.dma_start(cpk[:], cpack)
    id_sb = cpk[:, 0:128]
    w_sb = cpk[:, 128:640]     # [128, D*DI]
    m2_sb = cpk[:, 640:896]    # [128, DI*DI]

    # ---- constants with no DMA dependency ----
    ones_bf = const.tile([128, 128], F16, tag="ones_bf")
    nc.gpsimd.memset(ones_bf[:], 1.0 / O)
    zeros_bf = const.tile([128, 128], F16, tag="zeros_bf")
    nc.gpsimd.memset(zeros_bf[:], 0.0)
    half_sb = const.tile([128, B * J], F16, tag="half_sb")
    nc.gpsimd.memset(half_sb[:], 0.5)

    # xsp: padded/permuted squashed x, layout (j, b, w32); zero the pad lanes
    xsp = big.tile([128, J * 128], F16, tag="xsp")
    nc.gpsimd.memset(
        xsp[:].rearrange("p (j b w) -> p j b w", j=J, b=B)[:, :, :, DI:], 0.0
    )

    # act table preload (Exp; the only table this kernel ever loads)
    actpre = const.tile([128, 1], F32, tag="actpre")
    nc.vector.memset(actpre[:], 0.0)
    nc.scalar.activation(actpre[:], actpre[:], AF.Exp)

    # ---- PE warmup into the t psum bank (cleared later by start=True) ----
    tps2 = psumT.tile([128, 2 * B * DI], F32, tag="tps2")
    if WARMUP_MMS:
        for _ in range(WARMUP_MMS):
            nc.tensor.matmul(tps2[:, 0:64], lhsT=zeros_bf[:], rhs=zeros_bf[:, :64],
                             start=True, stop=True, skip_group_check=True)

    # ---- squash: n2 on DVE, sqrt on gpsimd ----
    xsq = big.tile([128, B * J * DI], F16, tag="xsq")
    nc.vector.tensor_mul(xsq[:], xr[:], xr[:])
    n2x = small.tile([128, B * J], F16, tag="n2x")
    with nc.allow_low_precision("fp16 n2 fine for 2e-2 gate"):
        nc.vector.reduce_sum(n2x[:], xsq[:].rearrange("p (r i) -> p r i", i=DI),
                             axis=mybir.AxisListType.X)
    gx = small.tile([128, B * J], F16, tag="gx")
    squash_scale(gx[:], n2x[:], "sq")

    # xs written directly into padded layout, chunked by 4 strips so the
    # transposes pipeline behind the multiplies
    xsT = big.tile([128, J * 128], F16, tag="xsT")
    xsp_v = xsp[:].rearrange("p (j b w) -> p j b w", j=J, b=B)
    xr_v = xr[:].rearrange("p (b j i) -> p j b i", b=B, j=J)
    gx_v = gx[:].rearrange("p (b j) -> p j b", b=B, j=J)
    for c in range(4):
        sl = slice(c * 4, c * 4 + 4)
        nc.vector.tensor_mul(
            xsp_v[:, sl, :, :DI],
            xr_v[:, sl],
            gx_v[:, sl].unsqueeze(3).broadcast_to([128, 4, B, DI]),
        )
        tpX = psumX.tile([128, 512], F16, tag="tpX")
        for jl in range(4):
            j = c * 4 + jl
            nc.tensor.transpose(tpX[:, jl * 128:(jl + 1) * 128],
                                xsp[:, j * 128:(j + 1) * 128], id_sb)
        if c % 2 == 0:
            nc.scalar.copy(xsT[:, c * 512:(c + 1) * 512], tpX[:])
        else:
            nc.vector.tensor_copy(xsT[:, c * 512:(c + 1) * 512], tpX[:])

    if DEBUG_DUMP == "xsT":
        dump_stop(xsT[:, 0:64], "xsT block j=0 cols 0:64")
        return

    # ---- persistent state ----
    # e layout [p, (j, pr, bl, o)]: 128-col (j, pr) slices are the t lhsT
    e_bf = big.tile([128, J * 2 * 128], F16, tag="e_bf")
    xz_bf = big.tile([128, J * B * DI], F16, tag="xz_bf")   # (j, b, i)
    z_sb = small.tile([128, J * B], F16, tag="z_sb")        # (j, pr, bl)
    rz_sb = small.tile([128, J * B], F16, tag="rz_sb")
    trc2 = big.tile([128, 256], F16, tag="trc2")
    wv_pad = const.tile([128, 2 * 32], F16, tag="wv_pad")   # (z, ii32)
    nc.gpsimd.memset(wv_pad[:], 0.0)
    wv0f = const.tile([128, 2 * DI], F32, tag="wv0f")       # (z, i)

    for it in range(3):
        if it == 0:
            # ---- t0 = (1/64) sum_n xs: one 64-col MM per strip ----
            for j in range(J):
                nc.tensor.matmul(
                    tps2[:, 0:B * DI],
                    lhsT=ones_bf[:],
                    rhs=xsp_v[:, j, :, :DI],
                    start=(j == 0),
                    stop=(j == J - 1),
                    skip_group_check=True,
                )
        else:
            # ---- logits + e + xz + t, chunked by 4 strips ----
            first_t = True
            for c in range(4):
                Lc = psumL.tile([128, 4 * 256], F32, tag="Lc")
                if DEBUG_DUMP == f"L{it}pre" and c == 0:
                    nc.tensor.matmul(Lc[:, 0:128], lhsT=zeros_bf[:], rhs=ones_bf[:],
                                     start=True, stop=True, skip_group_check=True)
                    dump_stop(Lc[:, 0:64], "Lc zeroed")
                    return
                if DEBUG_DUMP == f"L{it}pairB" and c == 0:
                    nc.tensor.matmul(Lc[:, 0:128], lhsT=xsT[0:64, 0:128],
                                     rhs=trc2[0:64, 0:128],
                                     start=True, stop=True, tile_position=(0, 0))
                    nc.tensor.matmul(Lc[:, 512:640], lhsT=xsT[64:128, 0:128],
                                     rhs=trc2[64:128, 128:256],
                                     start=True, stop=True, tile_position=(64, 0))
                    dump_stop(Lc[:, 512:576], "pairB diff bank")
                    return
                if DEBUG_DUMP == f"L{it}pairC" and c == 0:
                    nc.tensor.matmul(Lc[:, 0:128], lhsT=xsT[0:64, 0:128],
                                     rhs=trc2[0:64, 0:128],
                                     start=True, stop=True, tile_position=(0, 0))
                    nc.tensor.matmul(Lc[:, 128:256], lhsT=xsT[0:64, 128:256],
                                     rhs=trc2[0:64, 0:128],
                                     start=True, stop=True, tile_position=(0, 0))
                    dump_stop(Lc[:, 128:192], "pairC same position")
                    return
                if DEBUG_DUMP == f"L{it}pair" and c == 0:
                    nc.tensor.matmul(
                        Lc[:, 0:128],
                        lhsT=xsT[0:64, 0:128],
                        rhs=trc2[0:64, 0:128],
                        start=True, stop=True, tile_position=(0, 0),
                    )
                    nc.tensor.matmul(
                        Lc[:, 128:256],
                        lhsT=xsT[64:128, 0:128],
                        rhs=trc2[64:128, 128:256],
                        start=True, stop=True, tile_position=(64, 0),
                    )
                    dump_stop(Lc[:, 128:192], "pair of pair MMs")
                    return
                if DEBUG_DUMP == f"L{it}two" and c == 0:
                    nc.tensor.matmul(
                        Lc[:, 128:256],
                        lhsT=xsT[64:128, 0:128],
                        rhs=trc2[64:128, 128:256],
                        start=True, stop=True, tile_position=(64, 0),
                    )
                    dump_stop(Lc[:, 128:192], "pr1 pair MM")
                    return
                if DEBUG_DUMP == f"L{it}one" and c == 0:
                    nc.tensor.matmul(
                        Lc[:, 0:128],
                        lhsT=xsT[0:64, 0:128],
                        rhs=trc2[0:64, 0:128],
                        start=True, stop=True, tile_position=(0, 0),
                    )
                    dump_stop(Lc[:, 0:64], "one pair MM")
                    return
                # bank = pr: every PSUM bank only ever sees ONE PE
                # tile_position (two positions in one bank wedge the device)
                for pr in range(2):
                    for jl in range(4):
                        j = c * 4 + jl
                        nc.tensor.matmul(
                            Lc[:, (pr * 4 + jl) * 128:(pr * 4 + jl + 1) * 128],
                            lhsT=xsT[pr * 64:(pr + 1) * 64, j * 128:(j + 1) * 128],
                            rhs=trc2[pr * 64:(pr + 1) * 64, pr * 128:(pr + 1) * 128],
                            start=True,
                            stop=True,
                            tile_position=(pr * 64, 0),
                        )
                if DEBUG_DUMP == f"L{it}" and c == 0:
                    dump_stop(Lc[:, 0:64], f"L chunk0 it={it}")
                    return
                ec = e_bf[:, c * 1024:(c + 1) * 1024]
                nc.scalar.activation(ec, Lc[:], AF.Exp)
                with nc.allow_low_precision("fp16 z/rz fine for 2e-2 gate"):
                    nc.vector.reduce_sum(
                        z_sb[:, c * 16:(c + 1) * 16],
                        ec.rearrange("p (g o) -> p g o", o=O),
                        axis=mybir.AxisListType.X)
                    nc.vector.reciprocal(rz_sb[:, c * 16:(c + 1) * 16],
                                         z_sb[:, c * 16:(c + 1) * 16])
                sl = slice(c * 4, c * 4 + 4)
                xz_v5 = xz_bf[:].rearrange("p (j pr bl i) -> p j pr bl i",
                                           j=J, pr=2, bl=2, i=DI)[:, sl]
                xsp_v5 = xsp[:].rearrange("p (j pr bl w) -> p j pr bl w",
                                          j=J, pr=2, bl=2)[:, sl, :, :, :DI]
                for pr in range(2):
                    nc.vector.tensor_mul(
                        xz_v5[:, :, pr],
                        xsp_v5[:, :, pr],
                        rz_sb[:, c * 16 + pr * 8:c * 16 + (pr + 1) * 8]
                        .rearrange("p (j bl) -> p j bl", j=4, bl=2)
                        .unsqueeze(3).broadcast_to([128, 4, 2, DI]),
                    )
            # bank-wide clear; both pr regions accumulate afterwards
            nc.tensor.matmul(tps2[:], lhsT=zeros_bf[:], rhs=ones_bf[:],
                             start=True, stop=False, skip_group_check=True)
            for j in range(J):
                c2, jl = j // 4, j % 4
                for pr in range(2):
                    eslice = ((c2 * 2 + pr) * 4 + jl) * 128
                    nc.tensor.matmul(
                        tps2[:, pr * 64:(pr + 1) * 64],
                        lhsT=e_bf[:, eslice:eslice + 128],
                        rhs=xz_bf[:, j * 64:(j + 1) * 64],
                        start=False,
                        stop=(j == J - 1 and pr == 1),
                        skip_group_check=True,
                    )

        if DEBUG_DUMP == f"t{it}":
            dump_stop(tps2[:, 0:64], f"tps2 it={it}")
            return

        # ---- t_sb [128, (z, i)]: rows 0:64 = (b0, b2), rows 64:128 = (b1, b3)
        t_sb = small.tile([128, 2 * DI], F16, tag="t_sb")
        if it == 0:
            # single chunk holds all 4 batches (every row valid)
            nc.vector.tensor_copy(
                t_sb[0:64].rearrange("p (z i) -> p z i", z=2),
                tps2[0:64, 0:64].rearrange("p (c i) -> p c i", c=4)[:, 0::2],
            )
            nc.vector.tensor_copy(
                t_sb[64:128].rearrange("p (z i) -> p z i", z=2),
                tps2[64:128, 0:64].rearrange("p (c i) -> p c i", c=4)[:, 1::2],
            )
        else:
            # b0 @ pr0 col 0, b2 @ pr1 col 32 (stride 96 from col 0)
            nc.vector.tensor_copy(
                t_sb[0:64].rearrange("p (z i) -> p z i", z=2),
                tps2[0:64].rearrange("p (g i) -> p g i", g=8)[:, 0::6][:, 0:2],
            )
            # b1 @ pr0 col 16, b3 @ pr1 col 48 (stride 96 from col 16)
            nc.vector.tensor_copy(
                t_sb[64:128].rearrange("p (z i) -> p z i", z=2),
                tps2[64:128].rearrange("p (g i) -> p g i", g=8)[:, 1::6][:, 0:2],
            )

        if it < 2:
            # ---- small stage in [128, (z, i)] ----
            qm = small.tile([128, 2 * DI * DI], F16, tag="qm")
            nc.vector.tensor_mul(
                qm[:].rearrange("p (z i k) -> p z i k", z=2, i=DI),
                m2_sb.rearrange("p (i k) -> p i k", k=DI).unsqueeze(1).broadcast_to([128, 2, DI, DI]),
                t_sb[:].rearrange("p (z k) -> p z k", z=2).unsqueeze(2).broadcast_to([128, 2, DI, DI]),
            )
            q_t = small.tile([128, 2 * DI], F16, tag="q_t")
            with nc.allow_low_precision("fp16 q_t fine"):
                nc.vector.reduce_sum(q_t[:], qm[:].rearrange("p (r k) -> p r k", k=DI),
                                     axis=mybir.AxisListType.X)
            scr = small.tile([128, 2 * DI], F16, tag="scr")
            nc.vector.tensor_mul(scr[:], q_t[:], t_sb[:])
            n2t = small.tile([128, 2], F32, tag="n2t")
            nc.vector.reduce_sum(n2t[:], scr[:].rearrange("p (z i) -> p z i", z=2),
                                 axis=mybir.AxisListType.X)
            h = small.tile([128, 2], F32, tag="h")
            squash_scale(h[:], n2t[:], "h")
            wvv = wv_pad[:].rearrange("p (z w) -> p z w", z=2)[:, :, :DI]
            if it == 0:
                nc.vector.tensor_mul(
                    wv0f[:].rearrange("p (z i) -> p z i", z=2),
                    q_t[:].rearrange("p (z i) -> p z i", z=2),
                    h[:].unsqueeze(2).broadcast_to([128, 2, DI]),
                )
                nc.vector.tensor_copy(wvv, wv0f[:].rearrange("p (z i) -> p z i", z=2))
            else:
                hq = small.tile([128, 2 * DI], F32, tag="hq")
                nc.vector.tensor_mul(
                    hq[:].rearrange("p (z i) -> p z i", z=2),
                    q_t[:].rearrange("p (z i) -> p z i", z=2),
                    h[:].unsqueeze(2).broadcast_to([128, 2, DI]),
                )
                nc.vector.tensor_add(wvv, hq[:].rearrange("p (z i) -> p z i", z=2),
                                     wv0f[:].rearrange("p (z i) -> p z i", z=2))
            if DEBUG_DUMP == f"wv{it}":
                dump_stop(wv_pad[:], f"wv_pad it={it}")
                return
            # ---- trc2: [pr*64 partitions, pr*128 cols] block-diag wv^T ----
            # zero the full staging tile with two zero-transposes, then place
            # the four 32x64 wv^T blocks (pad rows included) at:
            #   b0 rows 0:32 cols 0:64    | b1 rows 32:64  cols 64:128
            #   b2 rows 64:96 cols 128:192| b3 rows 96:128 cols 192:256
            trcp = psumW.tile([128, 256], F16, tag="trcp")
            for half in range(2):
                nc.tensor.matmul(trcp[:, half * 128:(half + 1) * 128],
                                 lhsT=zeros_bf[:], rhs=id_sb,
                                 is_transpose=True, skip_group_check=True)
            # wv_pad layout: rows 0:64 z:(b0,b2), rows 64:128 z:(b1,b3)
            for bb in range(B):
                rhalf = bb % 2       # partition half of wv_pad
                z = bb // 2          # which 32-col z block of wv_pad
                nc.tensor.matmul(
                    trcp[bb * 32:(bb + 1) * 32, bb * 64:(bb + 1) * 64],
                    lhsT=wv_pad[rhalf * 64:(rhalf + 1) * 64, z * 32:(z + 1) * 32],
                    rhs=id_sb[rhalf * 64:(rhalf + 1) * 64, rhalf * 64:(rhalf + 1) * 64],
                    is_transpose=True,
                    skip_group_check=True,
                    tile_position=(rhalf * 64, (bb * 32) % 128),
                )
            nc.vector.tensor_copy(trc2[:], trcp[:])
            if DEBUG_DUMP == f"trc{it}":
                dump_stop(trc2[0:64, 0:64], f"trc2 rows0:64 cols 0:64 it={it}")
                return
        else:
            # ---- final: v = h * (W @ t) in [128, (z, d)] ----
            sm = small.tile([128, 2 * D * DI], F16, tag="sm")
            nc.vector.tensor_mul(
                sm[:].rearrange("p (z d i) -> p z d i", z=2, d=D),
                w_sb.rearrange("p (d i) -> p d i", i=DI).unsqueeze(1).broadcast_to([128, 2, D, DI]),
                t_sb[:].rearrange("p (z i) -> p z i", z=2).unsqueeze(2).broadcast_to([128, 2, D, DI]),
            )
            s_sb = small.tile([128, 2 * D], F32, tag="s_sb")
            nc.vector.reduce_sum(s_sb[:], sm[:].rearrange("p (r i) -> p r i", i=DI),
                                 axis=mybir.AxisListType.X)
            s2 = small.tile([128, 2 * D], F32, tag="s2")
            nc.vector.tensor_mul(s2[:], s_sb[:], s_sb[:])
            n2v = small.tile([128, 2], F32, tag="n2v")
            nc.vector.reduce_sum(n2v[:], s2[:].rearrange("p (z d) -> p z d", z=2),
                                 axis=mybir.AxisListType.X)
            hv = small.tile([128, 2], F32, tag="hv")
            squash_scale(hv[:], n2v[:], "hv")
            v_sb = small.tile([128, 2 * D], F32, tag="v_sb")
            nc.vector.tensor_mul(
                v_sb[:].rearrange("p (z d) -> p z d", z=2),
                s_sb[:].rearrange("p (z d) -> p z d", z=2),
                hv[:].unsqueeze(2).broadcast_to([128, 2, D]),
            )
            # b = 2z + b2:  vout[b] = v_sb[b2*64:(b2+1)*64, z*32:(z+1)*32]
            nc.sync.dma_start(
                vout.rearrange("(z b2) o d -> (b2 o) z d", z=2),
                v_sb[:].rearrange("p (z d) -> p z d", z=2),
            )
    ctx.close()


_CACHE = {}


def _get_module():
    if "nc" not in _CACHE:
        nc = bacc.Bacc("TRN2", target_bir_lowering=False, debug=False,
                       enable_asserts=False, num_devices=N_CORES)
        with tile.TileContext(nc) as tc:
            build_kernel(nc, tc)
        nc.compile()
        _CACHE["nc"] = nc
    return _CACHE["nc"]


def _host_inputs(input_vectors, weight_matrix):
    W0 = np.asarray(weight_matrix, dtype=np.float32)[0]          # [O, D, DI]
    M2 = np.einsum("odi,odj->oij", W0, W0).astype(np.float32)    # [O, DI, DI]
    wrep = np.tile(W0.reshape(O, D * DI), (2, 1)).astype(np.float16)
    m2rep = np.tile(M2.reshape(O, DI * DI), (2, 1)).astype(np.float16)
    ident = np.eye(128, dtype=np.float16)
    cpack = np.ascontiguousarray(
        np.concatenate([ident, wrep, m2rep], axis=1).astype(np.float16))
    x16 = np.ascontiguousarray(np.asarray(input_vectors).astype(np.float16))
    in_maps = []
    for c in range(N_CORES):
        in_maps.append({
            "x": np.ascontiguousarray(x16[c * B:(c + 1) * B]),
            "cpack": cpack,
        })
    return in_maps


def run(input_vectors, weight_matrix, trace=False, tmpdir=None):
    nc = _get_module()
    in_maps = _host_inputs(input_vectors, weight_matrix)
    res = run_bass_kernel_spmd(
        nc, in_maps, core_ids=list(range(N_CORES)), trace=trace, tmpdir=tmpdir
    )
    out = np.concatenate([res.results[c]["vout"] for c in range(N_CORES)], axis=0)
    return out.astype(np.float32), res


def kernel(input_vectors, weight_matrix):
    out, _ = run(input_vectors, weight_matrix, trace=False)
    return out


# revision 19
# speedup vs baseline: 1.4022x; 1.4022x over previous
"""CapsuleLayer (dynamic routing, 3 iterations) Trainium2 Bass kernel — v3.

Full inputs:  input_vectors [32, 2048, 16] f32, weight_matrix [1, 64, 32, 16] f32
Full output:  [32, 64, 32] f32

Sharding: data-parallel over batch; each of 8 NeuronCores processes 4 batches.
No collectives.

v3 changes vs v2 (67.6us):
  - ZERO act-table switches: sqrt runs on GpSimd via tensor_scalar(pow, 0.5);
    the scalar engine only ever uses Exp (+Copy), all in one resident table.
    (v2 paid 9 x 1283ns ACT_TABLE_LOADs for the ln/exp-based sqrt.)
  - x is cast to fp16 on host: half the DMA bytes, and 2-byte DVE ops.
  - x DMA issued first; all other constants packed into ONE dram tensor.
  - squash square+reduce on DVE (fp16, 2x modes) instead of scalar SQUARE.
  - squashed x written directly into the padded/permuted xsp layout (no
    separate pad-copy); transposes batched 4-per-PSUM-tile with single copies.
  - logits matmuls batch-paired via block-diag rhs: 32 MMs of 128 cols per
    iteration instead of 64, chunked 4 strips at a time over PSUM so EXP and
    the Z-reduce pipeline behind the MMs.
  - Z-reduce / reciprocal with fp16 outputs (DVE 2x path).
  - t-matmul streams all 4 batches per (strip, pair) MM: 32 MMs of 64 cols.
  - trc (wv^T blocks) built with 2 zeroing transposes + 4 block transposes +
    one PSUM->SBUF copy.
  - output written with a single fused DMA.

n-to-SBUF mapping: n = p*16 + j  (p = partition, j = strip 0..15).
t/psum layout: tps2 [128, (pr, b, i)]; valid rows: b even 0:64, b odd 64:128.
small stage in [128-part, (z, i)]: rows 0:64 hold (b0,b2), 64:128 (b1,b3).
Iteration-2 logits rhs built from wv0+wv1 (linearity) -> no PSUM carry-over.
"""

import os

os.environ.setdefault("MYCRO_LOCAL_CACHE", "1")

import numpy as np

import concourse.bass as bass
import concourse.tile as tile
from concourse import bacc, mybir
from concourse.bass_utils import run_bass_kernel_spmd

AF = mybir.ActivationFunctionType
ALU = mybir.AluOpType
F32 = mybir.dt.float32
F16 = mybir.dt.float16

N_CORES = 8
B = 4          # batches per core
N = 2048       # input capsules
O = 64         # output capsules
DI = 16        # input capsule dim
D = 32         # output capsule dim
J = 16         # n-strips per batch (n = p*16 + j)
EPS = 0.5

WARMUP_MMS = int(os.environ.get("CAPS_WARMUP_MMS", "60"))
SQRT_MODE = os.environ.get("CAPS_SQRT", "gpstt")
LOGITS_MODE = os.environ.get("CAPS_LOGITS", "pair")
DEBUG_DUMP = os.environ.get("CAPS_DEBUG_DUMP", "")


def build_kernel(nc: bass.Bass, tc: tile.TileContext):
    from contextlib import ExitStack
    ctx = ExitStack()
    x = nc.dram_tensor("x", [B, N, DI], F16, kind="ExternalInput").ap()
    cpack = nc.dram_tensor("cpack", [128, 896], F16, kind="ExternalInput").ap()
    vout = nc.dram_tensor("vout", [B, O, D], F32, kind="ExternalOutput").ap()

    const = ctx.enter_context(tc.tile_pool(name="const", bufs=1))
    big = ctx.enter_context(tc.tile_pool(name="big", bufs=1))
    small = ctx.enter_context(tc.tile_pool(name="small", bufs=2))
    psumT = ctx.enter_context(tc.tile_pool(name="psumT", bufs=1, space="PSUM"))
    psumL = ctx.enter_context(tc.tile_pool(name="psumL", bufs=2, space="PSUM"))
    psumX = ctx.enter_context(tc.tile_pool(name="psumX", bufs=2, space="PSUM"))
    psumW = ctx.enter_context(tc.tile_pool(name="psumW", bufs=1, space="PSUM"))

    def squash_scale(out, n2, tag, engine="gps"):
        # out = sqrt(n2)/(eps+n2).  engine="act": scalar Sqrt (needs the sqrt
        # act table resident).  engine="gps": GpSimd tensor_tensor pow (the
        # only pow the Pool ISA accepts; ~780ns fixed cost but no act-table
        # switch) in parallel with the add+reciprocal on DVE.
        s = small.tile(list(n2.shape), F32, tag=f"{tag}_s")
        cols = n2.shape[1]
        if engine == "act":
            nc.scalar.activation(s[:], n2, AF.Sqrt)
        else:
            nc.gpsimd.tensor_tensor(s[:], n2, half_sb[:, 0:cols], op=ALU.pow)
        d = small.tile(list(n2.shape), F32, tag=f"{tag}_d")
        nc.vector.tensor_scalar_add(d[:], n2, EPS)
        rd = small.tile(list(n2.shape), F32, tag=f"{tag}_rd")
        nc.vector.reciprocal(rd[:], d[:])
        nc.vector.tensor_mul(out, s[:], rd[:])

    def dump_stop(src, note=""):
        # stage src ([P, C] any dtype, P<=128, C<=64) into f32 and write vout
        stage = const.tile([128, 64], F32, tag="dumpstage")
        nc.gpsimd.memset(stage[:], 0.0)
        P, C = src.shape[0], src.shape[1]
        nc.vector.tensor_copy(stage[0:P, 0:C], src)
        nc.sync.dma_start(
            vout.rearrange("(z b2) o d -> (b2 o) z d", z=2),
            stage[:].rearrange("p (z d) -> p z d", z=2),
        )
        ctx.close()

    # ---- input DMA first (largest transfer), then constants ----
    xr = big.tile([128, B * J * DI], F16, tag="xr")
    nc.sync.dma_start(
        xr[:].rearrange("p (b j i) -> p b j i", b=B, j=J),
        x.rearrange("b (p j) i -> p b j i", p=128),
    )
    cpk = const.tile([128, 896], F16, tag="cpk")
    nc.sync.dma_start(cpk[:], cpack)
    id_sb = cpk[:, 0:128]
    w_sb = cpk[:, 128:640]     # [128, D*DI]
    m2_sb = cpk[:, 640:896]    # [128, DI*DI]

    # ---- constants with no DMA dependency ----
    ones_bf = const.tile([128, 128], F16, tag="ones_bf")
    nc.gpsimd.memset(ones_bf[:], 1.0 / O)
    zeros_bf = const.tile([128, 128], F16, tag="zeros_bf")
    nc.gpsimd.memset(zeros_bf[:], 0.0)
    half_sb = const.tile([128, B * J], F16, tag="half_sb")
    nc.gpsimd.memset(half_sb[:], 0.5)

    # xsp: padded/permuted squashed x, layout (j, b, w32); zero the pad lanes
    xsp = big.tile([128, J * 128], F16, tag="xsp")
    nc.gpsimd.memset(
        xsp[:].rearrange("p (j b w) -> p j b w", j=J, b=B)[:, :, :, DI:], 0.0
    )

    # act table preload: Sqrt first (used by the head squash); the Exp table
    # is prefetched by a dummy activation later, behind the t0/transposes
    actpre = const.tile([128, 1], F32, tag="actpre")
    nc.vector.memset(actpre[:], 0.0)
    nc.scalar.activation(actpre[:], actpre[:], AF.Sqrt)

    # ---- PE warmup into the t psum bank (cleared later by start=True) ----
    tps2 = psumT.tile([128, 2 * B * DI], F32, tag="tps2")
    if WARMUP_MMS:
        for _ in range(WARMUP_MMS):
            nc.tensor.matmul(tps2[:, 0:64], lhsT=zeros_bf[:], rhs=zeros_bf[:, :64],
                             start=True, stop=True, skip_group_check=True)

    # ---- squash: n2 on DVE, sqrt on gpsimd ----
    xsq = big.tile([128, B * J * DI], F16, tag="xsq")
    nc.vector.tensor_mul(xsq[:], xr[:], xr[:])
    n2x = small.tile([128, B * J], F32, tag="n2x")
    nc.vector.reduce_sum(n2x[:], xsq[:].rearrange("p (r i) -> p r i", i=DI),
                         axis=mybir.AxisListType.X)
    gx = small.tile([128, B * J], F16, tag="gx")
    squash_scale(gx[:], n2x[:], "sq", engine="act")

    # xs written directly into padded layout, chunked by 4 strips so the
    # transposes pipeline behind the multiplies
    xsT = big.tile([128, J * 128], F16, tag="xsT")
    xsp_v = xsp[:].rearrange("p (j b w) -> p j b w", j=J, b=B)
    xr_v = xr[:].rearrange("p (b j i) -> p j b i", b=B, j=J)
    gx_v = gx[:].rearrange("p (b j) -> p j b", b=B, j=J)
    trcp = psumW.tile([128, 256], F16, tag="trcp")
    for c in range(4):
        sl = slice(c * 4, c * 4 + 4)
        nc.vector.tensor_mul(
            xsp_v[:, sl, :, :DI],
            xr_v[:, sl],
            gx_v[:, sl].unsqueeze(3).broadcast_to([128, 4, B, DI]),
        )
        # t0 accumulation (does not need xsT) so small stage 0 can start
        # while the transposes/copies still run
        for jl in range(4):
            j = c * 4 + jl
            nc.tensor.matmul(
                tps2[:, 0:B * DI],
                lhsT=ones_bf[:],
                rhs=xsp_v[:, j, :, :DI],
                start=(j == 0),
                stop=(j == J - 1),
                skip_group_check=True,
            )
        tpX = psumX.tile([128, 512], F16, tag="tpX")
        for jl in range(4):
            j = c * 4 + jl
            nc.tensor.transpose(tpX[:, jl * 128:(jl + 1) * 128],
                                xsp[:, j * 128:(j + 1) * 128], id_sb)
        if c % 2 == 0:
            nc.scalar.copy(xsT[:, c * 512:(c + 1) * 512], tpX[:])
        else:
            nc.vector.tensor_copy(xsT[:, c * 512:(c + 1) * 512], tpX[:])
    # prefetch the Exp act table (load hides behind t0-tail / small stage 0)
    nc.scalar.activation(actpre[:], actpre[:], AF.Exp)
    # zero the trc staging tile once; per-iteration transposes only
    # overwrite the four wv^T blocks
    for half in range(2):
        nc.tensor.matmul(trcp[:, half * 128:(half + 1) * 128],
                         lhsT=zeros_bf[:], rhs=id_sb,
                         is_transpose=True, skip_group_check=True)

    if DEBUG_DUMP == "xsT":
        dump_stop(xsT[:, 0:64], "xsT block j=0 cols 0:64")
        return

    # ---- persistent state ----
    # e layout [p, (j, pr, bl, o)]: 128-col (j, pr) slices are the t lhsT
    e_bf = big.tile([128, J * 2 * 128], F16, tag="e_bf")
    xz_bf = big.tile([128, J * B * DI], F16, tag="xz_bf")   # (j, b, i)
    z_sb = small.tile([128, J * B], F32, tag="z_sb")        # (pr-major per chunk)
    rz_sb = small.tile([128, J * B], F32, tag="rz_sb")
    trc2 = big.tile([128, 256], F16, tag="trc2")
    wv_pad = const.tile([128, 2 * 32], F16, tag="wv_pad")   # (z, ii32)
    nc.gpsimd.memset(wv_pad[:], 0.0)
    wv0f = const.tile([128, 2 * DI], F32, tag="wv0f")       # (z, i)

    for it in range(3):
        if it == 0:
            pass  # t0 already accumulated during the squash phase
        else:
            # ---- logits + e + xz + t, chunked by 4 strips ----
            first_t = True
            for c in range(4):
                Lc = psumL.tile([128, 4 * 256], F32, tag="Lc")
                if DEBUG_DUMP == f"L{it}pre" and c == 0:
                    nc.tensor.matmul(Lc[:, 0:128], lhsT=zeros_bf[:], rhs=ones_bf[:],
                                     start=True, stop=True, skip_group_check=True)
                    dump_stop(Lc[:, 0:64], "Lc zeroed")
                    return
                if DEBUG_DUMP == f"L{it}pairB" and c == 0:
                    nc.tensor.matmul(Lc[:, 0:128], lhsT=xsT[0:64, 0:128],
                                     rhs=trc2[0:64, 0:128],
                                     start=True, stop=True, tile_position=(0, 0))
                    nc.tensor.matmul(Lc[:, 512:640], lhsT=xsT[64:128, 0:128],
                                     rhs=trc2[64:128, 128:256],
                                     start=True, stop=True, tile_position=(64, 0))
                    dump_stop(Lc[:, 512:576], "pairB diff bank")
                    return
                if DEBUG_DUMP == f"L{it}pairC" and c == 0:
                    nc.tensor.matmul(Lc[:, 0:128], lhsT=xsT[0:64, 0:128],
                                     rhs=trc2[0:64, 0:128],
                                     start=True, stop=True, tile_position=(0, 0))
                    nc.tensor.matmul(Lc[:, 128:256], lhsT=xsT[0:64, 128:256],
                                     rhs=trc2[0:64, 0:128],
                                     start=True, stop=True, tile_position=(0, 0))
                    dump_stop(Lc[:, 128:192], "pairC same position")
                    return
                if DEBUG_DUMP == f"L{it}pair" and c == 0:
                    nc.tensor.matmul(
                        Lc[:, 0:128],
                        lhsT=xsT[0:64, 0:128],
                        rhs=trc2[0:64, 0:128],
                        start=True, stop=True, tile_position=(0, 0),
                    )
                    nc.tensor.matmul(
                        Lc[:, 128:256],
                        lhsT=xsT[64:128, 0:128],
                        rhs=trc2[64:128, 128:256],
                        start=True, stop=True, tile_position=(64, 0),
                    )
                    dump_stop(Lc[:, 128:192], "pair of pair MMs")
                    return
                if DEBUG_DUMP == f"L{it}two" and c == 0:
                    nc.tensor.matmul(
                        Lc[:, 128:256],
                        lhsT=xsT[64:128, 0:128],
                        rhs=trc2[64:128, 128:256],
                        start=True, stop=True, tile_position=(64, 0),
                    )
                    dump_stop(Lc[:, 128:192], "pr1 pair MM")
                    return
                if DEBUG_DUMP == f"L{it}one" and c == 0:
                    nc.tensor.matmul(
                        Lc[:, 0:128],
                        lhsT=xsT[0:64, 0:128],
                        rhs=trc2[0:64, 0:128],
                        start=True, stop=True, tile_position=(0, 0),
                    )
                    dump_stop(Lc[:, 0:64], "one pair MM")
                    return
                # bank = pr: every PSUM bank only ever sees ONE PE
                # tile_position (two positions in one bank wedge the device)
                for pr in range(2):
                    for jl in range(4):
                        j = c * 4 + jl
                        nc.tensor.matmul(
                            Lc[:, (pr * 4 + jl) * 128:(pr * 4 + jl + 1) * 128],
                            lhsT=xsT[pr * 64:(pr + 1) * 64, j * 128:(j + 1) * 128],
                            rhs=trc2[pr * 64:(pr + 1) * 64, pr * 128:(pr + 1) * 128],
                            start=True,
                            stop=True,
                            tile_position=(pr * 64, 0),
                        )
                if DEBUG_DUMP == f"L{it}" and c == 0:
                    dump_stop(Lc[:, 0:64], f"L chunk0 it={it}")
                    return
                ec = e_bf[:, c * 1024:(c + 1) * 1024]
                nc.scalar.activation(ec, Lc[:], AF.Exp)
                nc.vector.reduce_sum(
                    z_sb[:, c * 16:(c + 1) * 16],
                    ec.rearrange("p (g o) -> p g o", o=O),
                    axis=mybir.AxisListType.X)
                nc.vector.reciprocal(rz_sb[:, c * 16:(c + 1) * 16],
                                     z_sb[:, c * 16:(c + 1) * 16])
                sl = slice(c * 4, c * 4 + 4)
                xz_v5 = xz_bf[:].rearrange("p (j pr bl i) -> p j pr bl i",
                                           j=J, pr=2, bl=2, i=DI)[:, sl]
                xsp_v5 = xsp[:].rearrange("p (j pr bl w) -> p j pr bl w",
                                          j=J, pr=2, bl=2)[:, sl, :, :, :DI]
                for pr in range(2):
                    nc.vector.tensor_mul(
                        xz_v5[:, :, pr],
                        xsp_v5[:, :, pr],
                        rz_sb[:, c * 16 + pr * 8:c * 16 + (pr + 1) * 8]
                        .rearrange("p (j bl) -> p j bl", j=4, bl=2)
                        .unsqueeze(3).broadcast_to([128, 4, 2, DI]),
                    )
            # bank-wide clear; both pr regions accumulate afterwards
            nc.tensor.matmul(tps2[:], lhsT=zeros_bf[:], rhs=ones_bf[:],
                             start=True, stop=False, skip_group_check=True)
            for j in range(J):
                c2, jl = j // 4, j % 4
                for pr in range(2):
                    eslice = ((c2 * 2 + pr) * 4 + jl) * 128
                    nc.tensor.matmul(
                        tps2[:, pr * 64:(pr + 1) * 64],
                        lhsT=e_bf[:, eslice:eslice + 128],
                        rhs=xz_bf[:, j * 64:(j + 1) * 64],
                        start=False,
                        stop=(j == J - 1 and pr == 1),
                        skip_group_check=True,
                    )

        if DEBUG_DUMP == f"t{it}":
            dump_stop(tps2[:, 0:64], f"tps2 it={it}")
            return

        # ---- t_sb [128, (z, i)]: rows 0:64 = (b0, b2), rows 64:128 = (b1, b3)
        t_sb = small.tile([128, 2 * DI], F16, tag="t_sb")
        if it == 0:
            # single chunk holds all 4 batches (every row valid)
            nc.vector.tensor_copy(
                t_sb[0:64].rearrange("p (z i) -> p z i", z=2),
                tps2[0:64, 0:64].rearrange("p (c i) -> p c i", c=4)[:, 0::2],
            )
            nc.vector.tensor_copy(
                t_sb[64:128].rearrange("p (z i) -> p z i", z=2),
                tps2[64:128, 0:64].rearrange("p (c i) -> p c i", c=4)[:, 1::2],
            )
        else:
            # b0 @ pr0 col 0, b2 @ pr1 col 32 (stride 96 from col 0)
            nc.vector.tensor_copy(
                t_sb[0:64].rearrange("p (z i) -> p z i", z=2),
                tps2[0:64].rearrange("p (g i) -> p g i", g=8)[:, 0::6][:, 0:2],
            )
            # b1 @ pr0 col 16, b3 @ pr1 col 48 (stride 96 from col 16)
            nc.vector.tensor_copy(
                t_sb[64:128].rearrange("p (z i) -> p z i", z=2),
                tps2[64:128].rearrange("p (g i) -> p g i", g=8)[:, 1::6][:, 0:2],
            )

        if it < 2:
            # ---- small stage in [128, (z, i)] ----
            qm = small.tile([128, 2 * DI * DI], F16, tag="qm")
            nc.vector.tensor_mul(
                qm[:].rearrange("p (z i k) -> p z i k", z=2, i=DI),
                m2_sb.rearrange("p (i k) -> p i k", k=DI).unsqueeze(1).broadcast_to([128, 2, DI, DI]),
                t_sb[:].rearrange("p (z k) -> p z k", z=2).unsqueeze(2).broadcast_to([128, 2, DI, DI]),
            )
            q_t = small.tile([128, 2 * DI], F32, tag="q_t")
            nc.vector.reduce_sum(q_t[:], qm[:].rearrange("p (r k) -> p r k", k=DI),
                                 axis=mybir.AxisListType.X)
            scr = small.tile([128, 2 * DI], F32, tag="scr")
            nc.vector.tensor_mul(scr[:], q_t[:], t_sb[:])
            n2t = small.tile([128, 2], F32, tag="n2t")
            nc.vector.reduce_sum(n2t[:], scr[:].rearrange("p (z i) -> p z i", z=2),
                                 axis=mybir.AxisListType.X)
            h = small.tile([128, 2], F32, tag="h")
            squash_scale(h[:], n2t[:], "h")
            wvv = wv_pad[:].rearrange("p (z w) -> p z w", z=2)[:, :, :DI]
            if it == 0:
                nc.vector.tensor_mul(
                    wv0f[:].rearrange("p (z i) -> p z i", z=2),
                    q_t[:].rearrange("p (z i) -> p z i", z=2),
                    h[:].unsqueeze(2).broadcast_to([128, 2, DI]),
                )
                nc.vector.tensor_copy(wvv, wv0f[:].rearrange("p (z i) -> p z i", z=2))
            else:
                hq = small.tile([128, 2 * DI], F32, tag="hq")
                nc.vector.tensor_mul(
                    hq[:].rearrange("p (z i) -> p z i", z=2),
                    q_t[:].rearrange("p (z i) -> p z i", z=2),
                    h[:].unsqueeze(2).broadcast_to([128, 2, DI]),
                )
                nc.vector.tensor_add(wvv, hq[:].rearrange("p (z i) -> p z i", z=2),
                                     wv0f[:].rearrange("p (z i) -> p z i", z=2))
            if DEBUG_DUMP == f"wv{it}":
                dump_stop(wv_pad[:], f"wv_pad it={it}")
                return
            # ---- trc2: [pr*64 partitions, pr*128 cols] block-diag wv^T ----
            # zero the full staging tile with two zero-transposes, then place
            # the four 32x64 wv^T blocks (pad rows included) at:
            #   b0 rows 0:32 cols 0:64    | b1 rows 32:64  cols 64:128
            #   b2 rows 64:96 cols 128:192| b3 rows 96:128 cols 192:256
            # wv_pad layout: rows 0:64 z:(b0,b2), rows 64:128 z:(b1,b3)
            for bb in range(B):
                rhalf = bb % 2       # partition half of wv_pad
                z = bb // 2          # which 32-col z block of wv_pad
                nc.tensor.matmul(
                    trcp[bb * 32:(bb + 1) * 32, bb * 64:(bb + 1) * 64],
                    lhsT=wv_pad[rhalf * 64:(rhalf + 1) * 64, z * 32:(z + 1) * 32],
                    rhs=id_sb[rhalf * 64:(rhalf + 1) * 64, rhalf * 64:(rhalf + 1) * 64],
                    is_transpose=True,
                    skip_group_check=True,
                    tile_position=(rhalf * 64, (bb * 32) % 128),
                )
            nc.vector.tensor_copy(trc2[:], trcp[:])
            if DEBUG_DUMP == f"trc{it}":
                dump_stop(trc2[0:64, 0:64], f"trc2 rows0:64 cols 0:64 it={it}")
                return
        else:
            # ---- final: v = h * (W @ t) in [128, (z, d)] ----
            sm = small.tile([128, 2 * D * DI], F16, tag="sm")
            nc.vector.tensor_mul(
                sm[:].rearrange("p (z d i) -> p z d i", z=2, d=D),
                w_sb.rearrange("p (d i) -> p d i", i=DI).unsqueeze(1).broadcast_to([128, 2, D, DI]),
                t_sb[:].rearrange("p (z i) -> p z i", z=2).unsqueeze(2).broadcast_to([128, 2, D, DI]),
            )
            s_sb = small.tile([128, 2 * D], F32, tag="s_sb")
            nc.vector.reduce_sum(s_sb[:], sm[:].rearrange("p (r i) -> p r i", i=DI),
                                 axis=mybir.AxisListType.X)
            s2 = small.tile([128, 2 * D], F32, tag="s2")
            nc.vector.tensor_mul(s2[:], s_sb[:], s_sb[:])
            n2v = small.tile([128, 2], F32, tag="n2v")
            nc.vector.reduce_sum(n2v[:], s2[:].rearrange("p (z d) -> p z d", z=2),
                                 axis=mybir.AxisListType.X)
            hv = small.tile([128, 2], F32, tag="hv")
            squash_scale(hv[:], n2v[:], "hv")
            v_sb = small.tile([128, 2 * D], F32, tag="v_sb")
            nc.vector.tensor_mul(
                v_sb[:].rearrange("p (z d) -> p z d", z=2),
                s_sb[:].rearrange("p (z d) -> p z d", z=2),
                hv[:].unsqueeze(2).broadcast_to([128, 2, D]),
            )
            # b = 2z + b2:  vout[b] = v_sb[b2*64:(b2+1)*64, z*32:(z+1)*32]
            nc.sync.dma_start(
                vout.rearrange("(z b2) o d -> (b2 o) z d", z=2),
                v_sb[:].rearrange("p (z d) -> p z d", z=2),
            )
    ctx.close()


_CACHE = {}


def _get_module():
    if "nc" not in _CACHE:
        nc = bacc.Bacc("TRN2", target_bir_lowering=False, debug=False,
                       enable_asserts=False, num_devices=N_CORES)
        with tile.TileContext(nc) as tc:
            build_kernel(nc, tc)
        nc.compile()
        _CACHE["nc"] = nc
    return _CACHE["nc"]


def _host_inputs(input_vectors, weight_matrix):
    W0 = np.asarray(weight_matrix, dtype=np.float32)[0]          # [O, D, DI]
    M2 = np.einsum("odi,odj->oij", W0, W0).astype(np.float32)    # [O, DI, DI]
    wrep = np.tile(W0.reshape(O, D * DI), (2, 1)).astype(np.float16)
    m2rep = np.tile(M2.reshape(O, DI * DI), (2, 1)).astype(np.float16)
    ident = np.eye(128, dtype=np.float16)
    cpack = np.ascontiguousarray(
        np.concatenate([ident, wrep, m2rep], axis=1).astype(np.float16))
    x16 = np.ascontiguousarray(np.asarray(input_vectors).astype(np.float16))
    in_maps = []
    for c in range(N_CORES):
        in_maps.append({
            "x": np.ascontiguousarray(x16[c * B:(c + 1) * B]),
            "cpack": cpack,
        })
    return in_maps


def run(input_vectors, weight_matrix, trace=False, tmpdir=None):
    nc = _get_module()
    in_maps = _host_inputs(input_vectors, weight_matrix)
    res = run_bass_kernel_spmd(
        nc, in_maps, core_ids=list(range(N_CORES)), trace=trace, tmpdir=tmpdir
    )
    out = np.concatenate([res.results[c]["vout"] for c in range(N_CORES)], axis=0)
    return out.astype(np.float32), res


def kernel(input_vectors, weight_matrix):
    out, _ = run(input_vectors, weight_matrix, trace=False)
    return out


# revision 20
# speedup vs baseline: 1.4215x; 1.0138x over previous
"""CapsuleLayer (dynamic routing, 3 iterations) Trainium2 Bass kernel — v3.

Full inputs:  input_vectors [32, 2048, 16] f32, weight_matrix [1, 64, 32, 16] f32
Full output:  [32, 64, 32] f32

Sharding: data-parallel over batch; each of 8 NeuronCores processes 4 batches.
No collectives.

v3 changes vs v2 (67.6us):
  - ZERO act-table switches: sqrt runs on GpSimd via tensor_scalar(pow, 0.5);
    the scalar engine only ever uses Exp (+Copy), all in one resident table.
    (v2 paid 9 x 1283ns ACT_TABLE_LOADs for the ln/exp-based sqrt.)
  - x is cast to fp16 on host: half the DMA bytes, and 2-byte DVE ops.
  - x DMA issued first; all other constants packed into ONE dram tensor.
  - squash square+reduce on DVE (fp16, 2x modes) instead of scalar SQUARE.
  - squashed x written directly into the padded/permuted xsp layout (no
    separate pad-copy); transposes batched 4-per-PSUM-tile with single copies.
  - logits matmuls batch-paired via block-diag rhs: 32 MMs of 128 cols per
    iteration instead of 64, chunked 4 strips at a time over PSUM so EXP and
    the Z-reduce pipeline behind the MMs.
  - Z-reduce / reciprocal with fp16 outputs (DVE 2x path).
  - t-matmul streams all 4 batches per (strip, pair) MM: 32 MMs of 64 cols.
  - trc (wv^T blocks) built with 2 zeroing transposes + 4 block transposes +
    one PSUM->SBUF copy.
  - output written with a single fused DMA.

n-to-SBUF mapping: n = p*16 + j  (p = partition, j = strip 0..15).
t/psum layout: tps2 [128, (pr, b, i)]; valid rows: b even 0:64, b odd 64:128.
small stage in [128-part, (z, i)]: rows 0:64 hold (b0,b2), 64:128 (b1,b3).
Iteration-2 logits rhs built from wv0+wv1 (linearity) -> no PSUM carry-over.
"""

import os

os.environ.setdefault("MYCRO_LOCAL_CACHE", "1")

import numpy as np

import concourse.bass as bass
import concourse.tile as tile
from concourse import bacc, mybir
from concourse.bass_utils import run_bass_kernel_spmd

AF = mybir.ActivationFunctionType
ALU = mybir.AluOpType
F32 = mybir.dt.float32
F16 = mybir.dt.float16

N_CORES = 8
B = 4          # batches per core
N = 2048       # input capsules
O = 64         # output capsules
DI = 16        # input capsule dim
D = 32         # output capsule dim
J = 16         # n-strips per batch (n = p*16 + j)
EPS = 0.5

WARMUP_MMS = int(os.environ.get("CAPS_WARMUP_MMS", "60"))
SQRT_MODE = os.environ.get("CAPS_SQRT", "gpstt")
LOGITS_MODE = os.environ.get("CAPS_LOGITS", "pair")
DEBUG_DUMP = os.environ.get("CAPS_DEBUG_DUMP", "")


def build_kernel(nc: bass.Bass, tc: tile.TileContext):
    from contextlib import ExitStack
    ctx = ExitStack()
    x = nc.dram_tensor("x", [B, N, DI], F16, kind="ExternalInput").ap()
    cpack = nc.dram_tensor("cpack", [128, 896], F16, kind="ExternalInput").ap()
    vout = nc.dram_tensor("vout", [B, O, D], F32, kind="ExternalOutput").ap()

    const = ctx.enter_context(tc.tile_pool(name="const", bufs=1))
    big = ctx.enter_context(tc.tile_pool(name="big", bufs=1))
    small = ctx.enter_context(tc.tile_pool(name="small", bufs=2))
    psumT = ctx.enter_context(tc.tile_pool(name="psumT", bufs=1, space="PSUM"))
    psumL = ctx.enter_context(tc.tile_pool(name="psumL", bufs=2, space="PSUM"))
    psumX = ctx.enter_context(tc.tile_pool(name="psumX", bufs=2, space="PSUM"))
    psumW = ctx.enter_context(tc.tile_pool(name="psumW", bufs=1, space="PSUM"))

    def squash_scale(out, n2, tag, engine="gps"):
        # out = sqrt(n2)/(eps+n2).  engine="act": scalar Sqrt (needs the sqrt
        # act table resident).  engine="gps": GpSimd tensor_tensor pow (the
        # only pow the Pool ISA accepts; ~780ns fixed cost but no act-table
        # switch) in parallel with the add+reciprocal on DVE.
        s = small.tile(list(n2.shape), F32, tag=f"{tag}_s")
        cols = n2.shape[1]
        if engine == "act":
            nc.scalar.activation(s[:], n2, AF.Sqrt)
        else:
            nc.gpsimd.tensor_tensor(s[:], n2, half_sb[:, 0:cols], op=ALU.pow)
        d = small.tile(list(n2.shape), F32, tag=f"{tag}_d")
        nc.vector.tensor_scalar_add(d[:], n2, EPS)
        rd = small.tile(list(n2.shape), F32, tag=f"{tag}_rd")
        nc.vector.reciprocal(rd[:], d[:])
        nc.vector.tensor_mul(out, s[:], rd[:])

    def dump_stop(src, note=""):
        # stage src ([P, C] any dtype, P<=128, C<=64) into f32 and write vout
        stage = const.tile([128, 64], F32, tag="dumpstage")
        nc.gpsimd.memset(stage[:], 0.0)
        P, C = src.shape[0], src.shape[1]
        nc.vector.tensor_copy(stage[0:P, 0:C], src)
        nc.sync.dma_start(
            vout.rearrange("(z b2) o d -> (b2 o) z d", z=2),
            stage[:].rearrange("p (z d) -> p z d", z=2),
        )
        ctx.close()

    # ---- input DMA first (largest transfer, 4 j-chunks so the squash
    # pipeline can start on chunk 0 early), then constants ----
    xr = big.tile([128, B * J * DI], F16, tag="xr")
    xr_dma = xr[:].rearrange("p (b j i) -> p b j i", b=B, j=J)
    x_dma = x.rearrange("b (p j) i -> p b j i", p=128)
    for dc in range(4):
        nc.sync.dma_start(
            xr_dma[:, :, dc * 4:(dc + 1) * 4],
            x_dma[:, :, dc * 4:(dc + 1) * 4],
        )
    cpk = const.tile([128, 896], F16, tag="cpk")
    nc.sync.dma_start(cpk[:], cpack)
    id_sb = cpk[:, 0:128]
    w_sb = cpk[:, 128:640]     # [128, D*DI]
    m2_sb = cpk[:, 640:896]    # [128, DI*DI]

    # ---- constants with no DMA dependency ----
    ones_bf = const.tile([128, 128], F16, tag="ones_bf")
    nc.gpsimd.memset(ones_bf[:], 1.0 / O)
    zeros_bf = const.tile([128, 128], F16, tag="zeros_bf")
    nc.gpsimd.memset(zeros_bf[:], 0.0)
    half_sb = const.tile([128, B * J], F16, tag="half_sb")
    nc.gpsimd.memset(half_sb[:], 0.5)

    # xsp: padded/permuted squashed x, layout (j, b, w32); zero the pad lanes
    xsp = big.tile([128, J * 128], F16, tag="xsp")
    nc.gpsimd.memset(
        xsp[:].rearrange("p (j b w) -> p j b w", j=J, b=B)[:, :, :, DI:], 0.0
    )

    # act table preload: Sqrt first (used by the head squash); the Exp table
    # is prefetched by a dummy activation later, behind the t0/transposes
    actpre = const.tile([128, 1], F32, tag="actpre")
    nc.vector.memset(actpre[:], 0.0)
    nc.scalar.activation(actpre[:], actpre[:], AF.Sqrt)

    # ---- PE warmup into the t psum bank (cleared later by start=True) ----
    tps2 = psumT.tile([128, 2 * B * DI], F32, tag="tps2")
    if WARMUP_MMS:
        for _ in range(WARMUP_MMS):
            nc.tensor.matmul(tps2[:, 0:64], lhsT=zeros_bf[:], rhs=zeros_bf[:, :64],
                             start=True, stop=True, skip_group_check=True)

    # ---- squash, chunked by 4 strips: square+reduce on DVE, sqrt on ACT ----
    xsq = big.tile([128, B * J * DI], F16, tag="xsq")
    n2x = small.tile([128, B * J], F32, tag="n2x")
    gx = small.tile([128, B * J], F16, tag="gx")
    xsq_v = xsq[:].rearrange("p (b j i) -> p b j i", b=B, j=J)
    xr_vc = xr[:].rearrange("p (b j i) -> p b j i", b=B, j=J)
    n2x_v = n2x[:].rearrange("p (b j) -> p b j", b=B)
    gx_vc = gx[:].rearrange("p (b j) -> p b j", b=B)
    for dc in range(4):
        dsl = slice(dc * 4, dc * 4 + 4)
        nc.vector.tensor_mul(xsq_v[:, :, dsl], xr_vc[:, :, dsl], xr_vc[:, :, dsl])
        nc.vector.reduce_sum(n2x_v[:, :, dsl],
                             xsq_v[:, :, dsl],
                             axis=mybir.AxisListType.X)
        s_c = small.tile([128, B * 4], F32, tag="sq_s")
        nc.scalar.activation(s_c[:].rearrange("p (b j) -> p b j", b=B),
                             n2x_v[:, :, dsl], AF.Sqrt)
        d_c = small.tile([128, B * 4], F32, tag="sq_d")
        nc.vector.tensor_scalar_add(d_c[:].rearrange("p (b j) -> p b j", b=B),
                                    n2x_v[:, :, dsl], EPS)
        rd_c = small.tile([128, B * 4], F32, tag="sq_rd")
        nc.vector.reciprocal(rd_c[:], d_c[:])
        nc.vector.tensor_mul(gx_vc[:, :, dsl],
                             s_c[:].rearrange("p (b j) -> p b j", b=B),
                             rd_c[:].rearrange("p (b j) -> p b j", b=B))

    # xs written directly into padded layout, chunked by 4 strips so the
    # transposes pipeline behind the multiplies
    xsT = big.tile([128, J * 128], F16, tag="xsT")
    xsp_v = xsp[:].rearrange("p (j b w) -> p j b w", j=J, b=B)
    xr_v = xr[:].rearrange("p (b j i) -> p j b i", b=B, j=J)
    gx_v = gx[:].rearrange("p (b j) -> p j b", b=B, j=J)
    trcp = psumW.tile([128, 256], F16, tag="trcp")
    for c in range(4):
        sl = slice(c * 4, c * 4 + 4)
        nc.vector.tensor_mul(
            xsp_v[:, sl, :, :DI],
            xr_v[:, sl],
            gx_v[:, sl].unsqueeze(3).broadcast_to([128, 4, B, DI]),
        )
        # t0 accumulation (does not need xsT) so small stage 0 can start
        # while the transposes/copies still run
        for jl in range(4):
            j = c * 4 + jl
            nc.tensor.matmul(
                tps2[:, 0:B * DI],
                lhsT=ones_bf[:],
                rhs=xsp_v[:, j, :, :DI],
                start=(j == 0),
                stop=(j == J - 1),
                skip_group_check=True,
            )
        tpX = psumX.tile([128, 512], F16, tag="tpX")
        for jl in range(4):
            j = c * 4 + jl
            nc.tensor.transpose(tpX[:, jl * 128:(jl + 1) * 128],
                                xsp[:, j * 128:(j + 1) * 128], id_sb)
        if c % 2 == 0:
            nc.scalar.copy(xsT[:, c * 512:(c + 1) * 512], tpX[:])
        else:
            nc.vector.tensor_copy(xsT[:, c * 512:(c + 1) * 512], tpX[:])
    # prefetch the Exp act table (load hides behind t0-tail / small stage 0)
    nc.scalar.activation(actpre[:], actpre[:], AF.Exp)
    # zero the trc staging tile once; per-iteration transposes only
    # overwrite the four wv^T blocks
    for half in range(2):
        nc.tensor.matmul(trcp[:, half * 128:(half + 1) * 128],
                         lhsT=zeros_bf[:], rhs=id_sb,
                         is_transpose=True, skip_group_check=True)

    if DEBUG_DUMP == "xsT":
        dump_stop(xsT[:, 0:64], "xsT block j=0 cols 0:64")
        return

    # ---- persistent state ----
    # e layout [p, (j, pr, bl, o)]: 128-col (j, pr) slices are the t lhsT
    e_bf = big.tile([128, J * 2 * 128], F16, tag="e_bf")
    xz_bf = big.tile([128, J * B * DI], F16, tag="xz_bf")   # (j, b, i)
    z_sb = small.tile([128, J * B], F32, tag="z_sb")        # (pr-major per chunk)
    rz_sb = small.tile([128, J * B], F32, tag="rz_sb")
    trc2 = big.tile([128, 256], F16, tag="trc2")
    wv_pad = const.tile([128, 2 * 32], F16, tag="wv_pad")   # (z, ii32)
    nc.gpsimd.memset(wv_pad[:], 0.0)
    wv0f = const.tile([128, 2 * DI], F32, tag="wv0f")       # (z, i)

    for it in range(3):
        if it == 0:
            pass  # t0 already accumulated during the squash phase
        else:
            # ---- logits + e + xz + t, chunked by 4 strips ----
            first_t = True
            for c in range(4):
                Lc = psumL.tile([128, 4 * 256], F32, tag="Lc")
                if DEBUG_DUMP == f"L{it}pre" and c == 0:
                    nc.tensor.matmul(Lc[:, 0:128], lhsT=zeros_bf[:], rhs=ones_bf[:],
                                     start=True, stop=True, skip_group_check=True)
                    dump_stop(Lc[:, 0:64], "Lc zeroed")
                    return
                if DEBUG_DUMP == f"L{it}pairB" and c == 0:
                    nc.tensor.matmul(Lc[:, 0:128], lhsT=xsT[0:64, 0:128],
                                     rhs=trc2[0:64, 0:128],
                                     start=True, stop=True, tile_position=(0, 0))
                    nc.tensor.matmul(Lc[:, 512:640], lhsT=xsT[64:128, 0:128],
                                     rhs=trc2[64:128, 128:256],
                                     start=True, stop=True, tile_position=(64, 0))
                    dump_stop(Lc[:, 512:576], "pairB diff bank")
                    return
                if DEBUG_DUMP == f"L{it}pairC" and c == 0:
                    nc.tensor.matmul(Lc[:, 0:128], lhsT=xsT[0:64, 0:128],
                                     rhs=trc2[0:64, 0:128],
                                     start=True, stop=True, tile_position=(0, 0))
                    nc.tensor.matmul(Lc[:, 128:256], lhsT=xsT[0:64, 128:256],
                                     rhs=trc2[0:64, 0:128],
                                     start=True, stop=True, tile_position=(0, 0))
                    dump_stop(Lc[:, 128:192], "pairC same position")
                    return
                if DEBUG_DUMP == f"L{it}pair" and c == 0:
                    nc.tensor.matmul(
                        Lc[:, 0:128],
                        lhsT=xsT[0:64, 0:128],
                        rhs=trc2[0:64, 0:128],
                        start=True, stop=True, tile_position=(0, 0),
                    )
                    nc.tensor.matmul(
                        Lc[:, 128:256],
                        lhsT=xsT[64:128, 0:128],
                        rhs=trc2[64:128, 128:256],
                        start=True, stop=True, tile_position=(64, 0),
                    )
                    dump_stop(Lc[:, 128:192], "pair of pair MMs")
                    return
                if DEBUG_DUMP == f"L{it}two" and c == 0:
                    nc.tensor.matmul(
                        Lc[:, 128:256],
                        lhsT=xsT[64:128, 0:128],
                        rhs=trc2[64:128, 128:256],
                        start=True, stop=True, tile_position=(64, 0),
                    )
                    dump_stop(Lc[:, 128:192], "pr1 pair MM")
                    return
                if DEBUG_DUMP == f"L{it}one" and c == 0:
                    nc.tensor.matmul(
                        Lc[:, 0:128],
                        lhsT=xsT[0:64, 0:128],
                        rhs=trc2[0:64, 0:128],
                        start=True, stop=True, tile_position=(0, 0),
                    )
                    dump_stop(Lc[:, 0:64], "one pair MM")
                    return
                # bank = pr: every PSUM bank only ever sees ONE PE
                # tile_position (two positions in one bank wedge the device)
                for pr in range(2):
                    for jl in range(4):
                        j = c * 4 + jl
                        nc.tensor.matmul(
                            Lc[:, (pr * 4 + jl) * 128:(pr * 4 + jl + 1) * 128],
                            lhsT=xsT[pr * 64:(pr + 1) * 64, j * 128:(j + 1) * 128],
                            rhs=trc2[pr * 64:(pr + 1) * 64, pr * 128:(pr + 1) * 128],
                            start=True,
                            stop=True,
                            tile_position=(pr * 64, 0),
                        )
                if DEBUG_DUMP == f"L{it}" and c == 0:
                    dump_stop(Lc[:, 0:64], f"L chunk0 it={it}")
                    return
                ec = e_bf[:, c * 1024:(c + 1) * 1024]
                nc.scalar.activation(ec, Lc[:], AF.Exp)
                nc.vector.reduce_sum(
                    z_sb[:, c * 16:(c + 1) * 16],
                    ec.rearrange("p (g o) -> p g o", o=O),
                    axis=mybir.AxisListType.X)
                nc.vector.reciprocal(rz_sb[:, c * 16:(c + 1) * 16],
                                     z_sb[:, c * 16:(c + 1) * 16])
                sl = slice(c * 4, c * 4 + 4)
                xz_v5 = xz_bf[:].rearrange("p (j pr bl i) -> p j pr bl i",
                                           j=J, pr=2, bl=2, i=DI)[:, sl]
                xsp_v5 = xsp[:].rearrange("p (j pr bl w) -> p j pr bl w",
                                          j=J, pr=2, bl=2)[:, sl, :, :, :DI]
                for pr in range(2):
                    eng = nc.gpsimd if pr == 0 else nc.vector
                    eng.tensor_mul(
                        xz_v5[:, :, pr],
                        xsp_v5[:, :, pr],
                        rz_sb[:, c * 16 + pr * 8:c * 16 + (pr + 1) * 8]
                        .rearrange("p (j bl) -> p j bl", j=4, bl=2)
                        .unsqueeze(3).broadcast_to([128, 4, 2, DI]),
                    )
            # bank-wide clear; both pr regions accumulate afterwards
            nc.tensor.matmul(tps2[:], lhsT=zeros_bf[:], rhs=ones_bf[:],
                             start=True, stop=False, skip_group_check=True)
            for j in range(J):
                c2, jl = j // 4, j % 4
                for pr in range(2):
                    eslice = ((c2 * 2 + pr) * 4 + jl) * 128
                    nc.tensor.matmul(
                        tps2[:, pr * 64:(pr + 1) * 64],
                        lhsT=e_bf[:, eslice:eslice + 128],
                        rhs=xz_bf[:, j * 64:(j + 1) * 64],
                        start=False,
                        stop=(j == J - 1 and pr == 1),
                        skip_group_check=True,
                    )

        if DEBUG_DUMP == f"t{it}":
            dump_stop(tps2[:, 0:64], f"tps2 it={it}")
            return

        # ---- t_sb [128, (z, i)]: rows 0:64 = (b0, b2), rows 64:128 = (b1, b3)
        t_sb = small.tile([128, 2 * DI], F16, tag="t_sb")
        if it == 0:
            # single chunk holds all 4 batches (every row valid)
            nc.vector.tensor_copy(
                t_sb[0:64].rearrange("p (z i) -> p z i", z=2),
                tps2[0:64, 0:64].rearrange("p (c i) -> p c i", c=4)[:, 0::2],
            )
            nc.vector.tensor_copy(
                t_sb[64:128].rearrange("p (z i) -> p z i", z=2),
                tps2[64:128, 0:64].rearrange("p (c i) -> p c i", c=4)[:, 1::2],
            )
        else:
            # b0 @ pr0 col 0, b2 @ pr1 col 32 (stride 96 from col 0)
            nc.vector.tensor_copy(
                t_sb[0:64].rearrange("p (z i) -> p z i", z=2),
                tps2[0:64].rearrange("p (g i) -> p g i", g=8)[:, 0::6][:, 0:2],
            )
            # b1 @ pr0 col 16, b3 @ pr1 col 48 (stride 96 from col 16)
            nc.vector.tensor_copy(
                t_sb[64:128].rearrange("p (z i) -> p z i", z=2),
                tps2[64:128].rearrange("p (g i) -> p g i", g=8)[:, 1::6][:, 0:2],
            )

        if it < 2:
            # ---- small stage in [128, (z, i)] ----
            # keepalive: dummy MMs that read chain outputs keep the PE
            # clock boosted through this serial phase
            for _ in range(3):
                nc.tensor.matmul(tps2[:, 0:16], lhsT=zeros_bf[:], rhs=t_sb[:, 0:16],
                                 start=True, stop=True, skip_group_check=True)
            qm = small.tile([128, 2 * DI * DI], F16, tag="qm")
            nc.vector.tensor_mul(
                qm[:].rearrange("p (z i k) -> p z i k", z=2, i=DI),
                m2_sb.rearrange("p (i k) -> p i k", k=DI).unsqueeze(1).broadcast_to([128, 2, DI, DI]),
                t_sb[:].rearrange("p (z k) -> p z k", z=2).unsqueeze(2).broadcast_to([128, 2, DI, DI]),
            )
            q_t = small.tile([128, 2 * DI], F32, tag="q_t")
            nc.vector.reduce_sum(q_t[:], qm[:].rearrange("p (r k) -> p r k", k=DI),
                                 axis=mybir.AxisListType.X)
            scr = small.tile([128, 2 * DI], F32, tag="scr")
            nc.vector.tensor_mul(scr[:], q_t[:], t_sb[:])
            n2t = small.tile([128, 2], F32, tag="n2t")
            nc.vector.reduce_sum(n2t[:], scr[:].rearrange("p (z i) -> p z i", z=2),
                                 axis=mybir.AxisListType.X)
            for _ in range(4):
                nc.tensor.matmul(tps2[:, 0:16], lhsT=zeros_bf[:], rhs=qm[:, 0:16],
                                 start=True, stop=True, skip_group_check=True)
            h = small.tile([128, 2], F32, tag="h")
            squash_scale(h[:], n2t[:], "h")
            wvv = wv_pad[:].rearrange("p (z w) -> p z w", z=2)[:, :, :DI]
            if it == 0:
                nc.vector.tensor_mul(
                    wv0f[:].rearrange("p (z i) -> p z i", z=2),
                    q_t[:].rearrange("p (z i) -> p z i", z=2),
                    h[:].unsqueeze(2).broadcast_to([128, 2, DI]),
                )
                nc.vector.tensor_copy(wvv, wv0f[:].rearrange("p (z i) -> p z i", z=2))
            else:
                hq = small.tile([128, 2 * DI], F32, tag="hq")
                nc.vector.tensor_mul(
                    hq[:].rearrange("p (z i) -> p z i", z=2),
                    q_t[:].rearrange("p (z i) -> p z i", z=2),
                    h[:].unsqueeze(2).broadcast_to([128, 2, DI]),
                )
                nc.vector.tensor_add(wvv, hq[:].rearrange("p (z i) -> p z i", z=2),
                                     wv0f[:].rearrange("p (z i) -> p z i", z=2))
            if DEBUG_DUMP == f"wv{it}":
                dump_stop(wv_pad[:], f"wv_pad it={it}")
                return
            # ---- trc2: [pr*64 partitions, pr*128 cols] block-diag wv^T ----
            # zero the full staging tile with two zero-transposes, then place
            # the four 32x64 wv^T blocks (pad rows included) at:
            #   b0 rows 0:32 cols 0:64    | b1 rows 32:64  cols 64:128
            #   b2 rows 64:96 cols 128:192| b3 rows 96:128 cols 192:256
            for _ in range(4):
                nc.tensor.matmul(tps2[:, 0:16], lhsT=zeros_bf[:],
                                 rhs=wv_pad[:, 0:16],
                                 start=True, stop=True, skip_group_check=True)
            # wv_pad layout: rows 0:64 z:(b0,b2), rows 64:128 z:(b1,b3)
            for bb in range(B):
                rhalf = bb % 2       # partition half of wv_pad
                z = bb // 2          # which 32-col z block of wv_pad
                nc.tensor.matmul(
                    trcp[bb * 32:(bb + 1) * 32, bb * 64:(bb + 1) * 64],
                    lhsT=wv_pad[rhalf * 64:(rhalf + 1) * 64, z * 32:(z + 1) * 32],
                    rhs=id_sb[rhalf * 64:(rhalf + 1) * 64, rhalf * 64:(rhalf + 1) * 64],
                    is_transpose=True,
                    skip_group_check=True,
                    tile_position=(rhalf * 64, (bb * 32) % 128),
                )
            nc.vector.tensor_copy(trc2[:], trcp[:])
            if DEBUG_DUMP == f"trc{it}":
                dump_stop(trc2[0:64, 0:64], f"trc2 rows0:64 cols 0:64 it={it}")
                return
        else:
            # ---- final: v = h * (W @ t) in [128, (z, d)] ----
            sm = small.tile([128, 2 * D * DI], F16, tag="sm")
            nc.vector.tensor_mul(
                sm[:].rearrange("p (z d i) -> p z d i", z=2, d=D),
                w_sb.rearrange("p (d i) -> p d i", i=DI).unsqueeze(1).broadcast_to([128, 2, D, DI]),
                t_sb[:].rearrange("p (z i) -> p z i", z=2).unsqueeze(2).broadcast_to([128, 2, D, DI]),
            )
            s_sb = small.tile([128, 2 * D], F32, tag="s_sb")
            nc.vector.reduce_sum(s_sb[:], sm[:].rearrange("p (r i) -> p r i", i=DI),
                                 axis=mybir.AxisListType.X)
            s2 = small.tile([128, 2 * D], F32, tag="s2")
            nc.vector.tensor_mul(s2[:], s_sb[:], s_sb[:])
            n2v = small.tile([128, 2], F32, tag="n2v")
            nc.vector.reduce_sum(n2v[:], s2[:].rearrange("p (z d) -> p z d", z=2),
                                 axis=mybir.AxisListType.X)
            hv = small.tile([128, 2], F32, tag="hv")
            squash_scale(hv[:], n2v[:], "hv")
            v_sb = small.tile([128, 2 * D], F32, tag="v_sb")
            nc.vector.tensor_mul(
                v_sb[:].rearrange("p (z d) -> p z d", z=2),
                s_sb[:].rearrange("p (z d) -> p z d", z=2),
                hv[:].unsqueeze(2).broadcast_to([128, 2, D]),
            )
            # b = 2z + b2:  vout[b] = v_sb[b2*64:(b2+1)*64, z*32:(z+1)*32]
            nc.sync.dma_start(
                vout.rearrange("(z b2) o d -> (b2 o) z d", z=2),
                v_sb[:].rearrange("p (z d) -> p z d", z=2),
            )
    ctx.close()


_CACHE = {}


def _get_module():
    if "nc" not in _CACHE:
        nc = bacc.Bacc("TRN2", target_bir_lowering=False, debug=False,
                       enable_asserts=False, num_devices=N_CORES)
        with tile.TileContext(nc) as tc:
            build_kernel(nc, tc)
        nc.compile()
        _CACHE["nc"] = nc
    return _CACHE["nc"]


def _host_inputs(input_vectors, weight_matrix):
    W0 = np.asarray(weight_matrix, dtype=np.float32)[0]          # [O, D, DI]
    M2 = np.einsum("odi,odj->oij", W0, W0).astype(np.float32)    # [O, DI, DI]
    wrep = np.tile(W0.reshape(O, D * DI), (2, 1)).astype(np.float16)
    m2rep = np.tile(M2.reshape(O, DI * DI), (2, 1)).astype(np.float16)
    ident = np.eye(128, dtype=np.float16)
    cpack = np.ascontiguousarray(
        np.concatenate([ident, wrep, m2rep], axis=1).astype(np.float16))
    x16 = np.ascontiguousarray(np.asarray(input_vectors).astype(np.float16))
    in_maps = []
    for c in range(N_CORES):
        in_maps.append({
            "x": np.ascontiguousarray(x16[c * B:(c + 1) * B]),
            "cpack": cpack,
        })
    return in_maps


def run(input_vectors, weight_matrix, trace=False, tmpdir=None):
    nc = _get_module()
    in_maps = _host_inputs(input_vectors, weight_matrix)
    res = run_bass_kernel_spmd(
        nc, in_maps, core_ids=list(range(N_CORES)), trace=trace, tmpdir=tmpdir
    )
    out = np.concatenate([res.results[c]["vout"] for c in range(N_CORES)], axis=0)
    return out.astype(np.float32), res


def kernel(input_vectors, weight_matrix):
    out, _ = run(input_vectors, weight_matrix, trace=False)
    return out
